# revision 16
# baseline (speedup 1.0000x reference)
"""Trainium2 Bass kernel for AsymmetricWeightsDequantizer.

result = zero_point + weight * scale  (per [O, G] group, broadcast over GS)
         + svd_up @ svd_down          (rank-128 correction)

Sharding: output dim O split across 8 cores (1024 rows each), svd_down
replicated.

v7 structure (wide ops, simple 4-stage pipeline, tuned head/tail):
  per 1024-col block of each 128-row tile (8 blocks/tile):
   - DVE: ONE wide paged-broadcast tensor_tensor
         q[p,(g,j)] = w[p,(g,j)] * scale[p,g]   (scale AP has 0-stride
         over the 128 in-group cols -> 8 groups in one instruction)
   - PE:  psum = [up | z_hi | z_lo*16] @ [down | E | E/16]  -- fp8e4
          DoubleRow matmuls (contract 256 at 2 MACs/cell), then
          psum += I @ q  (fp16 identity accumulate)
   - ACT: ONE wide activation(Copy) psum -> out_sb fp16
   - DMA: w streamed in 2048-col quarters (first quarter ahead of the
     big consts so compute starts early); out drained in 2048-col
     quarters; out is fp16 (host upcasts), halving write traffic
"""

import numpy as np
import ml_dtypes

import concourse.bass as bass
import concourse.bacc as bacc
import concourse.mybir as mybir
import concourse.tile as tile
from concourse import bass_utils

O, G, GS = 8192, 64, 128
I = G * GS              # 8192
RANK = 128
NCORES = 8
OP = O // NCORES        # 1024 rows per core
NT = OP // 128          # 8 partition tiles per core
NBLK = 4                # 2048-col blocks per row tile
BLK = I // NBLK         # 2048
GPB = G // NBLK         # 16 groups per block
NPS = BLK // 512        # 512-col DR matmul slices per block

BF16 = ml_dtypes.bfloat16
FP8 = ml_dtypes.float8_e4m3fn
F32 = mybir.dt.float32
FP16 = mybir.dt.float16
F8 = mybir.dt.float8e4
U8 = mybir.dt.uint8

_cached_nc = None


def _build():
    global _cached_nc
    if _cached_nc is not None:
        return _cached_nc

    nc = bacc.Bacc("TRN2", target_bir_lowering=False, debug=False,
                   num_devices=NCORES)

    w_d = nc.dram_tensor("w", [OP, I], U8, kind="ExternalInput")
    sc_d = nc.dram_tensor("scale_r", [128, NT * G], F32, kind="ExternalInput")
    # stationary planes: [up | zp_hi/lo stack], fp8e4, channel-plane layout
    st_d = nc.dram_tensor("stat", [128, 2 * OP], F8, kind="ExternalInput")
    # moving planes: [down | E/E-over-16 stack]
    cb_d = nc.dram_tensor("comb", [128, 2 * I], F8, kind="ExternalInput")
    id_d = nc.dram_tensor("ident", [128, 128], FP16, kind="ExternalInput")
    out_d = nc.dram_tensor("out", [OP, I], FP16, kind="ExternalOutput")

    with tile.TileContext(nc) as tc:
        with (
            tc.tile_pool(name="const", bufs=1) as cpool,
            tc.tile_pool(name="wp", bufs=3) as wpool,
            tc.tile_pool(name="qp", bufs=3) as qpool,
            tc.tile_pool(name="outp", bufs=2) as opool,
            tc.tile_pool(name="ps", bufs=2, space="PSUM") as pspool,
        ):
            st_sb = cpool.tile([128, 2 * OP], F8)
            cb_sb = cpool.tile([128, 2 * I], F8)
            sc_sb = cpool.tile([128, NT * G], F32)
            id_sb = cpool.tile([128, 128], FP16)

            st3 = st_sb[:].rearrange("p (c m) -> p c m", c=2)
            cb3 = cb_sb[:].rearrange("p (c n) -> p c n", c=2)
            cb3_d = cb_d[:].rearrange("p (c n) -> p c n", c=2)

            # issue order tuned so the first DVE mul starts ASAP:
            # scale + first half of tile-0 weights lead, big consts behind
            nc.sync.dma_start(sc_sb[:], sc_d[:])
            nc.sync.dma_start(id_sb[:], id_d[:])

            for t in range(NT):
                w_sb = wpool.tile([128, I], U8)
                rows = slice(t * 128, (t + 1) * 128)
                if t == 0:
                    nc.sync.dma_start(w_sb[:, 0:BLK], w_d[rows, 0:BLK])
                    nc.sync.dma_start(st_sb[:], st_d[:])
                    nc.sync.dma_start(cb_sb[:], cb_d[:])
                    nc.sync.dma_start(w_sb[:, BLK:I], w_d[rows, BLK:I])
                else:
                    nc.sync.dma_start(w_sb[:], w_d[rows, :])
                out_sb = opool.tile([128, I], FP16)

                def paged_mul(eng, q_t, nb):
                    w3 = w_sb[:, nb * BLK:(nb + 1) * BLK].rearrange(
                        "p (g j) -> p g j", g=GPB)
                    q3 = q_t[:].rearrange("p (g j) -> p g j", g=GPB)
                    scb = sc_sb[:, t * G + nb * GPB:
                                t * G + (nb + 1) * GPB].unsqueeze(2)
                    eng.tensor_tensor(q3, w3,
                                      scb.broadcast_to((128, GPB, GS)),
                                      op=mybir.AluOpType.mult)

                for nb in range(NBLK):
                    ps = pspool.tile([128, BLK], F32)
                    q = qpool.tile([128, BLK], FP16)
                    paged_mul(nc.vector, q, nb)

                    # PE: fused svd+zp DoubleRow fp8 matmuls (contract 256)
                    for k in range(NPS):
                        n = nb * NPS + k
                        nc.tensor.matmul(
                            ps[:, k * 512:(k + 1) * 512],
                            st3[:, :, t * 128:(t + 1) * 128],
                            cb3[:, :, n * 512:(n + 1) * 512],
                            start=True, stop=False,
                            perf_mode=mybir.MatmulPerfMode.DoubleRow,
                        )
                    # PE: identity accumulate of q (fp16 moving max 512)
                    for k in range(NPS):
                        nc.tensor.matmul(
                            ps[:, k * 512:(k + 1) * 512],
                            id_sb[:],
                            q[:, k * 512:(k + 1) * 512],
                            start=False, stop=True,
                        )
                    # ACT: one wide copy psum -> fp16 out subtile
                    nc.scalar.activation(
                        out_sb[:, nb * BLK:(nb + 1) * BLK], ps[:],
                        mybir.ActivationFunctionType.Copy,
                        bias=0.0, scale=1.0)

                    if t == NT - 1 and nb in (1, 2):
                        cols = slice(0 if nb == 1 else I // 2,
                                     I // 2 if nb == 1 else I // 2 + BLK)
                        nc.scalar.dma_start(out_d[rows, cols],
                                            out_sb[:, cols])
                if t == NT - 1:
                    nc.scalar.dma_start(out_d[rows, I - BLK:I],
                                        out_sb[:, I - BLK:I])
                else:
                    nc.scalar.dma_start(out_d[rows, :], out_sb[:])

    nc.compile()
    _cached_nc = nc
    return nc


def _make_in_maps(weight, scale, zero_point, svd_up, svd_down):
    w = np.ascontiguousarray(weight.reshape(O, I)).astype(np.uint8)
    sc = np.ascontiguousarray(scale.reshape(O, G).astype(np.float32))
    zp = np.ascontiguousarray(zero_point.reshape(O, G).astype(np.float32))
    down8 = np.ascontiguousarray(svd_down).astype(FP8)       # [RANK, I]

    # group indicator planes: rows 0..63 = E (for zp_hi), 64..127 = E/16
    # (the lo channel is pre-scaled x16 so values stay in fp8 normal range)
    eb2 = np.zeros((128, I), dtype=np.float32)
    for g in range(G):
        eb2[g, g * GS:(g + 1) * GS] = 1.0
        eb2[G + g, g * GS:(g + 1) * GS] = 1.0 / 16.0
    comb = np.concatenate([down8.astype(np.float32), eb2],
                          axis=1).astype(FP8)                # [128, 2I]

    ident = np.eye(128, dtype=np.float16)

    in_maps = []
    for c in range(NCORES):
        sl = slice(c * OP, (c + 1) * OP)
        scr = np.ascontiguousarray(
            sc[sl].reshape(NT, 128, G).transpose(1, 0, 2).reshape(
                128, NT * G))
        z = zp[sl]                           # [OP, G] f32
        z_hi = z.astype(FP8)
        z_lo = ((z - z_hi.astype(np.float32)) * 16.0).astype(FP8)
        zeroT2 = np.concatenate([z_hi.T, z_lo.T], axis=0)    # [128, OP] fp8
        upT8 = np.ascontiguousarray(svd_up[sl].T).astype(FP8)
        stat = np.concatenate([upT8.astype(np.float32),
                               zeroT2.astype(np.float32)],
                              axis=1).astype(FP8)            # [128, 2*OP]
        in_maps.append({
            "w": np.ascontiguousarray(w[sl]),
            "scale_r": scr,
            "stat": np.ascontiguousarray(stat),
            "comb": np.ascontiguousarray(comb),
            "ident": ident,
        })
    return in_maps


def _run(in_maps, trace=False, **kwargs):
    nc = _build()
    return bass_utils.run_bass_kernel_spmd(
        nc, in_maps, core_ids=list(range(NCORES)), trace=trace, **kwargs)


def kernel(weight, scale, zero_point, svd_up, svd_down):
    in_maps = _make_in_maps(np.asarray(weight), np.asarray(scale),
                            np.asarray(zero_point), np.asarray(svd_up),
                            np.asarray(svd_down))
    res = _run(in_maps)
    out = np.concatenate([res.results[c]["out"] for c in range(NCORES)],
                         axis=0)
    return out.astype(np.float32)


# revision 17
# speedup vs baseline: 1.1951x; 1.1951x over previous
"""Trainium2 Bass kernel for AsymmetricWeightsDequantizer.

result = zero_point + weight * scale  (per [O, G] group, broadcast over GS)
         + svd_up @ svd_down          (rank-128 correction)

Sharding: output dim O split across 8 cores (1024 rows each), svd_down
replicated.

v7 structure (wide ops, simple 4-stage pipeline, tuned head/tail):
  per 1024-col block of each 128-row tile (8 blocks/tile):
   - DVE: ONE wide paged-broadcast tensor_tensor
         q[p,(g,j)] = w[p,(g,j)] * scale[p,g]   (scale AP has 0-stride
         over the 128 in-group cols -> 8 groups in one instruction)
   - PE:  psum = [up | z_hi | z_lo*16] @ [down | E | E/16]  -- fp8e4
          DoubleRow matmuls (contract 256 at 2 MACs/cell), then
          psum += I @ q  (fp16 identity accumulate)
   - ACT: ONE wide activation(Copy) psum -> out_sb fp16
   - DMA: w streamed in 2048-col quarters (first quarter ahead of the
     big consts so compute starts early); out drained in 2048-col
     quarters; out is fp16 (host upcasts), halving write traffic
"""

import numpy as np
import ml_dtypes

import concourse.bass as bass
import concourse.bacc as bacc
import concourse.mybir as mybir
import concourse.tile as tile
from concourse import bass_utils

O, G, GS = 8192, 64, 128
I = G * GS              # 8192
RANK = 128
NCORES = 8
OP = O // NCORES        # 1024 rows per core
NT = OP // 128          # 8 partition tiles per core
NBLK = 4                # 2048-col blocks per row tile
BLK = I // NBLK         # 2048
GPB = G // NBLK         # 16 groups per block
NPS = BLK // 512        # 512-col DR matmul slices per block

BF16 = ml_dtypes.bfloat16
FP8 = ml_dtypes.float8_e4m3fn
F32 = mybir.dt.float32
FP16 = mybir.dt.float16
F8 = mybir.dt.float8e4
U8 = mybir.dt.uint8

_cached_nc = None


def _build():
    global _cached_nc
    if _cached_nc is not None:
        return _cached_nc

    nc = bacc.Bacc("TRN2", target_bir_lowering=False, debug=False,
                   num_devices=NCORES)

    w_d = nc.dram_tensor("w", [OP, I], U8, kind="ExternalInput")
    sc_d = nc.dram_tensor("scale_r", [128, NT * G], F32, kind="ExternalInput")
    # stationary planes: [up | zp_hi/lo stack], fp8e4, channel-plane layout
    st_d = nc.dram_tensor("stat", [128, 2 * OP], F8, kind="ExternalInput")
    # moving planes: [down | E/E-over-16 stack]
    cb_d = nc.dram_tensor("comb", [128, 2 * I], F8, kind="ExternalInput")
    id_d = nc.dram_tensor("ident", [128, 128], FP16, kind="ExternalInput")
    out_d = nc.dram_tensor("out", [OP, I], FP16, kind="ExternalOutput")

    with tile.TileContext(nc) as tc:
        with (
            tc.tile_pool(name="const", bufs=1) as cpool,
            tc.tile_pool(name="wp", bufs=3) as wpool,
            tc.tile_pool(name="qp", bufs=3) as qpool,
            tc.tile_pool(name="outp", bufs=2) as opool,
            tc.tile_pool(name="ps", bufs=2, space="PSUM") as pspool,
        ):
            st_sb = cpool.tile([128, 2 * OP], F8)
            cb_sb = cpool.tile([128, 2 * I], F8)
            sc_sb = cpool.tile([128, NT * G], F32)
            id_sb = cpool.tile([128, 128], FP16)

            st3 = st_sb[:].rearrange("p (c m) -> p c m", c=2)
            cb3 = cb_sb[:].rearrange("p (c n) -> p c n", c=2)
            cb3_d = cb_d[:].rearrange("p (c n) -> p c n", c=2)

            # issue order tuned so the first DVE mul starts ASAP:
            # scale + first half of tile-0 weights lead, big consts behind
            nc.sync.dma_start(sc_sb[:], sc_d[:])
            nc.sync.dma_start(id_sb[:], id_d[:])

            for t in range(NT):
                w_sb = wpool.tile([128, I], U8)
                rows = slice(t * 128, (t + 1) * 128)
                if t == 0:
                    nc.sync.dma_start(w_sb[:, 0:I // 2], w_d[rows, 0:I // 2])
                    nc.sync.dma_start(st_sb[:], st_d[:])
                    nc.sync.dma_start(cb_sb[:], cb_d[:])
                    nc.sync.dma_start(w_sb[:, I // 2:I], w_d[rows, I // 2:I])
                else:
                    nc.sync.dma_start(w_sb[:], w_d[rows, :])
                out_sb = opool.tile([128, I], FP16)

                def paged_mul(eng, q_t, nb):
                    w3 = w_sb[:, nb * BLK:(nb + 1) * BLK].rearrange(
                        "p (g j) -> p g j", g=GPB)
                    q3 = q_t[:].rearrange("p (g j) -> p g j", g=GPB)
                    scb = sc_sb[:, t * G + nb * GPB:
                                t * G + (nb + 1) * GPB].unsqueeze(2)
                    eng.tensor_tensor(q3, w3,
                                      scb.broadcast_to((128, GPB, GS)),
                                      op=mybir.AluOpType.mult)

                for nb in range(NBLK):
                    ps = pspool.tile([128, BLK], F32)
                    q = qpool.tile([128, BLK], FP16)
                    paged_mul(nc.vector, q, nb)

                    # PE: fused svd+zp DoubleRow fp8 matmuls (contract 256)
                    for k in range(NPS):
                        n = nb * NPS + k
                        nc.tensor.matmul(
                            ps[:, k * 512:(k + 1) * 512],
                            st3[:, :, t * 128:(t + 1) * 128],
                            cb3[:, :, n * 512:(n + 1) * 512],
                            start=True, stop=False,
                            perf_mode=mybir.MatmulPerfMode.DoubleRow,
                        )
                    # PE: identity accumulate of q (fp16 moving max 512)
                    for k in range(NPS):
                        nc.tensor.matmul(
                            ps[:, k * 512:(k + 1) * 512],
                            id_sb[:],
                            q[:, k * 512:(k + 1) * 512],
                            start=False, stop=True,
                        )
                    # ACT: one wide copy psum -> fp16 out subtile
                    nc.scalar.activation(
                        out_sb[:, nb * BLK:(nb + 1) * BLK], ps[:],
                        mybir.ActivationFunctionType.Copy,
                        bias=0.0, scale=1.0)

                    if t == NT - 1 and nb == 1:
                        nc.scalar.dma_start(out_d[rows, 0:I // 2],
                                            out_sb[:, 0:I // 2])
                if t == NT - 1:
                    nc.scalar.dma_start(out_d[rows, I // 2:I],
                                        out_sb[:, I // 2:I])
                else:
                    nc.scalar.dma_start(out_d[rows, :], out_sb[:])

    nc.compile()
    _cached_nc = nc
    return nc


def _make_in_maps(weight, scale, zero_point, svd_up, svd_down):
    w = np.ascontiguousarray(weight.reshape(O, I)).astype(np.uint8)
    sc = np.ascontiguousarray(scale.reshape(O, G).astype(np.float32))
    zp = np.ascontiguousarray(zero_point.reshape(O, G).astype(np.float32))
    down8 = np.ascontiguousarray(svd_down).astype(FP8)       # [RANK, I]

    # group indicator planes: rows 0..63 = E (for zp_hi), 64..127 = E/16
    # (the lo channel is pre-scaled x16 so values stay in fp8 normal range)
    eb2 = np.zeros((128, I), dtype=np.float32)
    for g in range(G):
        eb2[g, g * GS:(g + 1) * GS] = 1.0
        eb2[G + g, g * GS:(g + 1) * GS] = 1.0 / 16.0
    comb = np.concatenate([down8.astype(np.float32), eb2],
                          axis=1).astype(FP8)                # [128, 2I]

    ident = np.eye(128, dtype=np.float16)

    in_maps = []
    for c in range(NCORES):
        sl = slice(c * OP, (c + 1) * OP)
        scr = np.ascontiguousarray(
            sc[sl].reshape(NT, 128, G).transpose(1, 0, 2).reshape(
                128, NT * G))
        z = zp[sl]                           # [OP, G] f32
        z_hi = z.astype(FP8)
        z_lo = ((z - z_hi.astype(np.float32)) * 16.0).astype(FP8)
        zeroT2 = np.concatenate([z_hi.T, z_lo.T], axis=0)    # [128, OP] fp8
        upT8 = np.ascontiguousarray(svd_up[sl].T).astype(FP8)
        stat = np.concatenate([upT8.astype(np.float32),
                               zeroT2.astype(np.float32)],
                              axis=1).astype(FP8)            # [128, 2*OP]
        in_maps.append({
            "w": np.ascontiguousarray(w[sl]),
            "scale_r": scr,
            "stat": np.ascontiguousarray(stat),
            "comb": np.ascontiguousarray(comb),
            "ident": ident,
        })
    return in_maps


def _run(in_maps, trace=False, **kwargs):
    nc = _build()
    return bass_utils.run_bass_kernel_spmd(
        nc, in_maps, core_ids=list(range(NCORES)), trace=trace, **kwargs)


def kernel(weight, scale, zero_point, svd_up, svd_down):
    in_maps = _make_in_maps(np.asarray(weight), np.asarray(scale),
                            np.asarray(zero_point), np.asarray(svd_up),
                            np.asarray(svd_down))
    res = _run(in_maps)
    out = np.concatenate([res.results[c]["out"] for c in range(NCORES)],
                         axis=0)
    return out.astype(np.float32)


# revision 19
# speedup vs baseline: 1.2218x; 1.0224x over previous
"""Trainium2 Bass kernel for AsymmetricWeightsDequantizer.

result = zero_point + weight * scale  (per [O, G] group, broadcast over GS)
         + svd_up @ svd_down          (rank-128 correction)

Sharding: output dim O split across 8 cores (1024 rows each), svd_down
replicated.  Measured: ~97 us HW exec (8 cores), rel_norm err ~3.9e-4.

Final structure -- wide ops only, per 2048-col block of each 128-row tile:
 - DVE: ONE wide paged-broadcast tensor_tensor
        q[p,(g,j)] = w[p,(g,j)] * scale[p,g]  (the scale operand is a 3D
        AP with 0-stride over the 128 in-group columns, so one
        instruction covers 16 groups; DVE is the saturated engine at
        ~2.28us per block)
 - PE:  psum = [up | z_hi | z_lo*16] @ [down | E | E/16] -- fp8e4
        DoubleRow matmuls (contract 256 = 128 svd + 64 zp_hi + 64 zp_lo
        at 2 MACs/cell), then psum += I @ q (fp16 identity accumulate)
 - ACT: ONE wide activation(Copy) psum -> out_sb fp16
 - DMA: weights uint8 (4x smaller than the int32 input), output fp16
        (host upcasts to fp32; halves write traffic, rel err ~2^-11).
        Output DMAs ride the ACT HWDGE ring so weight loads on the SP
        ring never queue behind them; tile-0 weights lead the big
        consts; the last tile drains in halves.
"""

import numpy as np
import ml_dtypes

import concourse.bass as bass
import concourse.bacc as bacc
import concourse.mybir as mybir
import concourse.tile as tile
from concourse import bass_utils

O, G, GS = 8192, 64, 128
I = G * GS              # 8192
RANK = 128
NCORES = 8
OP = O // NCORES        # 1024 rows per core
NT = OP // 128          # 8 partition tiles per core
NBLK = 4                # 2048-col blocks per row tile
BLK = I // NBLK         # 2048
GPB = G // NBLK         # 16 groups per block
NPS = BLK // 512        # 512-col DR matmul slices per block

BF16 = ml_dtypes.bfloat16
FP8 = ml_dtypes.float8_e4m3fn
F32 = mybir.dt.float32
FP16 = mybir.dt.float16
F8 = mybir.dt.float8e4
U8 = mybir.dt.uint8

_cached_nc = None


def _build():
    global _cached_nc
    if _cached_nc is not None:
        return _cached_nc

    nc = bacc.Bacc("TRN2", target_bir_lowering=False, debug=False,
                   num_devices=NCORES)

    w_d = nc.dram_tensor("w", [OP, I], U8, kind="ExternalInput")
    sc_d = nc.dram_tensor("scale_r", [128, NT * G], F32, kind="ExternalInput")
    # stationary planes: [up | zp_hi/lo stack], fp8e4, channel-plane layout
    st_d = nc.dram_tensor("stat", [128, 2 * OP], F8, kind="ExternalInput")
    # moving planes: [down | E/E-over-16 stack]
    cb_d = nc.dram_tensor("comb", [128, 2 * I], F8, kind="ExternalInput")
    id_d = nc.dram_tensor("ident", [128, 128], FP16, kind="ExternalInput")
    out_d = nc.dram_tensor("out", [OP, I], FP16, kind="ExternalOutput")

    with tile.TileContext(nc) as tc:
        with (
            tc.tile_pool(name="const", bufs=1) as cpool,
            tc.tile_pool(name="wp", bufs=3) as wpool,
            tc.tile_pool(name="qp", bufs=3) as qpool,
            tc.tile_pool(name="outp", bufs=2) as opool,
            tc.tile_pool(name="ps", bufs=2, space="PSUM") as pspool,
        ):
            st_sb = cpool.tile([128, 2 * OP], F8)
            cb_sb = cpool.tile([128, 2 * I], F8)
            sc_sb = cpool.tile([128, NT * G], F32)
            id_sb = cpool.tile([128, 128], FP16)

            st3 = st_sb[:].rearrange("p (c m) -> p c m", c=2)
            cb3 = cb_sb[:].rearrange("p (c n) -> p c n", c=2)
            cb3_d = cb_d[:].rearrange("p (c n) -> p c n", c=2)

            # issue order tuned so the first DVE mul starts ASAP:
            # scale + first half of tile-0 weights lead, big consts behind
            nc.sync.dma_start(sc_sb[:], sc_d[:])
            nc.sync.dma_start(id_sb[:], id_d[:])

            for t in range(NT):
                w_sb = wpool.tile([128, I], U8)
                rows = slice(t * 128, (t + 1) * 128)
                if t == 0:
                    nc.sync.dma_start(w_sb[:, 0:I // 2], w_d[rows, 0:I // 2])
                    nc.sync.dma_start(st_sb[:], st_d[:])
                    nc.sync.dma_start(cb3[:, :, 0:BLK], cb3_d[:, :, 0:BLK])
                    nc.sync.dma_start(w_sb[:, I // 2:I], w_d[rows, I // 2:I])
                    for j in range(1, NBLK):
                        nc.sync.dma_start(cb3[:, :, j * BLK:(j + 1) * BLK],
                                          cb3_d[:, :, j * BLK:(j + 1) * BLK])
                else:
                    nc.sync.dma_start(w_sb[:], w_d[rows, :])
                out_sb = opool.tile([128, I], FP16)

                def paged_mul(eng, q_t, nb):
                    w3 = w_sb[:, nb * BLK:(nb + 1) * BLK].rearrange(
                        "p (g j) -> p g j", g=GPB)
                    q3 = q_t[:].rearrange("p (g j) -> p g j", g=GPB)
                    scb = sc_sb[:, t * G + nb * GPB:
                                t * G + (nb + 1) * GPB].unsqueeze(2)
                    eng.tensor_tensor(q3, w3,
                                      scb.broadcast_to((128, GPB, GS)),
                                      op=mybir.AluOpType.mult)

                for nb in range(NBLK):
                    ps = pspool.tile([128, BLK], F32)
                    q = qpool.tile([128, BLK], FP16)
                    paged_mul(nc.vector, q, nb)
                    is_tt2 = (t == NT - 1 and nb == NBLK - 1)

                    # PE: fused svd+zp DoubleRow fp8 matmuls (contract 256)
                    for k in range(NPS):
                        n = nb * NPS + k
                        nc.tensor.matmul(
                            ps[:, k * 512:(k + 1) * 512],
                            st3[:, :, t * 128:(t + 1) * 128],
                            cb3[:, :, n * 512:(n + 1) * 512],
                            start=True, stop=(is_tt2 and k == NPS - 1),
                            perf_mode=mybir.MatmulPerfMode.DoubleRow,
                        )
                    if is_tt2:
                        # final block: DVE adds q+psum straight to out fp16,
                        # skipping the id matmul + ACT copy on the tail chain
                        nc.vector.tensor_tensor(
                            out_sb[:, nb * BLK:(nb + 1) * BLK], q[:], ps[:],
                            op=mybir.AluOpType.add)
                    else:
                        # PE: identity accumulate of q (fp16 moving max 512)
                        for k in range(NPS):
                            nc.tensor.matmul(
                                ps[:, k * 512:(k + 1) * 512],
                                id_sb[:],
                                q[:, k * 512:(k + 1) * 512],
                                start=False, stop=True,
                            )
                        # ACT: one wide copy psum -> fp16 out subtile
                        nc.scalar.activation(
                            out_sb[:, nb * BLK:(nb + 1) * BLK], ps[:],
                            mybir.ActivationFunctionType.Copy,
                            bias=0.0, scale=1.0)

                    if t == NT - 1 and nb == 1:
                        nc.scalar.dma_start(out_d[rows, 0:I // 2],
                                            out_sb[:, 0:I // 2])
                if t == NT - 1:
                    nc.scalar.dma_start(out_d[rows, I // 2:I],
                                        out_sb[:, I // 2:I])
                else:
                    nc.scalar.dma_start(out_d[rows, :], out_sb[:])

    nc.compile()
    _cached_nc = nc
    return nc


def _make_in_maps(weight, scale, zero_point, svd_up, svd_down):
    w = np.ascontiguousarray(weight.reshape(O, I)).astype(np.uint8)
    sc = np.ascontiguousarray(scale.reshape(O, G).astype(np.float32))
    zp = np.ascontiguousarray(zero_point.reshape(O, G).astype(np.float32))
    down8 = np.ascontiguousarray(svd_down).astype(FP8)       # [RANK, I]

    # group indicator planes: rows 0..63 = E (for zp_hi), 64..127 = E/16
    # (the lo channel is pre-scaled x16 so values stay in fp8 normal range)
    eb2 = np.zeros((128, I), dtype=np.float32)
    for g in range(G):
        eb2[g, g * GS:(g + 1) * GS] = 1.0
        eb2[G + g, g * GS:(g + 1) * GS] = 1.0 / 16.0
    comb = np.concatenate([down8.astype(np.float32), eb2],
                          axis=1).astype(FP8)                # [128, 2I]

    ident = np.eye(128, dtype=np.float16)

    in_maps = []
    for c in range(NCORES):
        sl = slice(c * OP, (c + 1) * OP)
        scr = np.ascontiguousarray(
            sc[sl].reshape(NT, 128, G).transpose(1, 0, 2).reshape(
                128, NT * G))
        z = zp[sl]                           # [OP, G] f32
        z_hi = z.astype(FP8)
        z_lo = ((z - z_hi.astype(np.float32)) * 16.0).astype(FP8)
        zeroT2 = np.concatenate([z_hi.T, z_lo.T], axis=0)    # [128, OP] fp8
        upT8 = np.ascontiguousarray(svd_up[sl].T).astype(FP8)
        stat = np.concatenate([upT8.astype(np.float32),
                               zeroT2.astype(np.float32)],
                              axis=1).astype(FP8)            # [128, 2*OP]
        in_maps.append({
            "w": np.ascontiguousarray(w[sl]),
            "scale_r": scr,
            "stat": np.ascontiguousarray(stat),
            "comb": np.ascontiguousarray(comb),
            "ident": ident,
        })
    return in_maps


def _run(in_maps, trace=False, **kwargs):
    nc = _build()
    return bass_utils.run_bass_kernel_spmd(
        nc, in_maps, core_ids=list(range(NCORES)), trace=trace, **kwargs)


def kernel(weight, scale, zero_point, svd_up, svd_down):
    in_maps = _make_in_maps(np.asarray(weight), np.asarray(scale),
                            np.asarray(zero_point), np.asarray(svd_up),
                            np.asarray(svd_down))
    res = _run(in_maps)
    out = np.concatenate([res.results[c]["out"] for c in range(NCORES)],
                         axis=0)
    return out.astype(np.float32)
